# revision 3
# baseline (speedup 1.0000x reference)
"""Trainium2 Bass kernel for nn_ContinuousEmbedding (histogram binning + distance-
weighted embedding mix).

Math: for each scalar x[b,f], the reference computes bucket index
idx = #{j in 1..63 : x > low[j]} and returns
    out[b,f,:] = sum_k weight[k,:] / (|idx-k|+1)  =  T[idx,:]
where T = S @ weight, S[i,k] = 1/(|i-k|+1) is a fixed 64x64 matrix.

T[idx] telescopes over the monotone compare vector G_j = 1[x > low[j]]:
    T[idx] = T[0] + sum_{j>=1} G_j * (T[j]-T[j-1])  =  G' @ V
with G'_0 = 1[x > low[0]] = 1[x > -inf] = 1 and V[0]=T[0], V[j]=T[j]-T[j-1].

Device kernel (data-parallel over 8 NeuronCores, batch-sharded):
  per 2048-token chunk:
    gpsimd.partition_broadcast : x row -> xb [64, 2048]          (bins on partitions)
    vector.tensor_scalar is_gt : sg = (xb > low_col) fp16 [64, 2048]
    16x tensor.matmul          : psum[128, 64m:64m+64] = sg[:,128m:128m+128].T @ V
    scalar.copy                : psum [128,1024] f32 -> sbuf
    sync.dma_start             : sbuf -> out rows (HWDGE)
V (64x64, fp16) is precomputed on the host from `weight`/`low` in float64; all
O(B*F*K) work (binning + the bfk,kd einsum contraction) runs on device.
"""

import sys

import numpy as np

for _p in ("/opt/trn_rl_repo",):
    if _p not in sys.path:
        sys.path.insert(0, _p)

import ml_dtypes

import concourse.bass as bass  # noqa: E402
import concourse.mybir as mybir  # noqa: E402
import concourse.tile as tile  # noqa: E402
from concourse import bacc  # noqa: E402
from concourse import bass_utils  # noqa: E402

B, F, K, D = 8192, 64, 64, 64
NCORES = 8
NTOK = (B // NCORES) * F          # 65536 tokens per core
CHUNK = 2048                      # tokens per pipeline chunk
NCHUNK = NTOK // CHUNK            # 32
MPC = CHUNK // 128                # matmuls per chunk (16)

FP16 = mybir.dt.float16
F32 = mybir.dt.float32


def build_tile_kernel(nc, tc, x_d, low_d, v_d, out_d):
    x_ap = x_d.ap().rearrange("(c n) -> c n", c=NCHUNK)          # [32, 2048]
    # out rows for chunk c, psum layout: partition q <-> token 128m+q, block m
    out_ap = out_d.ap().rearrange("(c m q) d -> c q m d", c=NCHUNK, m=MPC, q=128)

    with tc.tile_pool(name="cpool", bufs=1) as cpool:
        lowcol = cpool.tile([K, 1], F32)
        nc.sync.dma_start(out=lowcol[:], in_=low_d.ap())
        vtab = cpool.tile([K, D], FP16)
        nc.sync.dma_start(out=vtab[:], in_=v_d.ap())

        with (
            tc.tile_pool(name="wpool", bufs=2) as wpool,
            tc.tile_pool(name="opool", bufs=3) as opool,
            tc.tile_pool(name="ppool", bufs=2, space="PSUM") as ppool,
        ):
            for c in range(NCHUNK):
                xrow = wpool.tile([1, CHUNK], F32, tag="xrow", bufs=3)
                nc.sync.dma_start(out=xrow[:], in_=x_ap[c])
                xb = wpool.tile([K, CHUNK], F32, tag="xb")
                nc.gpsimd.partition_broadcast(xb[:], xrow[:])
                sg = wpool.tile([K, CHUNK], FP16, tag="sg")
                nc.vector.tensor_scalar(
                    out=sg[:],
                    in0=xb[:],
                    scalar1=lowcol[:],
                    scalar2=None,
                    op0=mybir.AluOpType.is_gt,
                )
                ps = ppool.tile([128, MPC * D], F32, tag="ps")
                for m in range(MPC):
                    nc.tensor.matmul(
                        out=ps[:, D * m : D * (m + 1)],
                        lhsT=sg[:, 128 * m : 128 * (m + 1)],
                        rhs=vtab[:],
                        start=True,
                        stop=True,
                    )
                ob = opool.tile([128, MPC * D], F32, tag="ob")
                nc.scalar.copy(out=ob[:], in_=ps[:])
                nc.sync.dma_start(
                    out=out_ap[c],
                    in_=ob[:].rearrange("q (m d) -> q m d", m=MPC),
                )


_CACHED_NC = None


def _get_nc():
    global _CACHED_NC
    if _CACHED_NC is None:
        nc = bacc.Bacc("TRN2", target_bir_lowering=False, debug=False)
        x_d = nc.dram_tensor("x", [NTOK], F32, kind="ExternalInput")
        low_d = nc.dram_tensor("lowcol", [K, 1], F32, kind="ExternalInput")
        v_d = nc.dram_tensor("vtab", [K, D], FP16, kind="ExternalInput")
        out_d = nc.dram_tensor("out", [NTOK, D], F32, kind="ExternalOutput")
        with tile.TileContext(nc) as tc:
            build_tile_kernel(nc, tc, x_d, low_d, v_d, out_d)
        nc.compile()
        _CACHED_NC = nc
    return _CACHED_NC


def make_host_tables(low, weight):
    """V [K, D] fp16 and low column [K,1] f32, computed in float64."""
    ar = np.arange(K)
    S = 1.0 / (np.abs(ar[:, None] - ar[None, :]) + 1.0)          # [K, K] f64
    T = S @ weight.astype(np.float64)                            # [K, D]
    V = np.empty_like(T)
    V[0] = T[0]
    V[1:] = T[1:] - T[:-1]
    vtab = V.astype(np.float16)
    lowcol = np.asarray(low, np.float32).reshape(K, 1)
    return lowcol, vtab


def kernel(x, low, high, weight):
    x = np.asarray(x, np.float32)
    weight = np.asarray(weight, np.float32)
    lowcol, vtab = make_host_tables(low, weight)

    nc = _get_nc()
    shards = x.reshape(NCORES, NTOK)
    in_maps = [
        {"x": np.ascontiguousarray(shards[i]), "lowcol": lowcol, "vtab": vtab}
        for i in range(NCORES)
    ]
    res = bass_utils.run_bass_kernel_spmd(nc, in_maps, core_ids=list(range(NCORES)))
    out = np.concatenate([res.results[i]["out"] for i in range(NCORES)], axis=0)
    return out.reshape(B, F, D)
